# revision 5
# baseline (speedup 1.0000x reference)
"""GroupEmbedding kernel for Trainium2 (8 NeuronCores, Bass/Tile).

  beh_emb      = item_table[behavior_item_ids] * behavior_counts[:,None]
  per_user_beh = segment_sum(beh_emb, behavior_user_ids, n_users)
  ue           = user_table[user_ids] * (user_ids != 0)
  per_user     = per_user_beh * ue
  out          = segment_sum(per_user, user_group_ids, num_groups)

Sharding: 512 windows of 128 users; 64 windows per core (data parallel on the
ragged behavior axis, windows are offset-aligned so each user lives on one
core).  Per window the behaviors are bucketed by item-table quarter (dma_gather
indices are int16) and streamed in 128-entry tiles: one dma_gather per
(window, quarter) run fetches the item rows, a fused DVE op builds the
count-scaled one-hot selection matrix, and PE matmuls accumulate per-user sums
in PSUM.  The window epilogue gathers the 128 user embeddings, multiplies them
in, and a second one-hot matmul reduces users into per-window group-rank slots
streamed to DRAM.  Host sums the slots across windows/cores (the cross-shard
psum) into the [num_groups, EMB] output.
"""

import sys

sys.path.insert(0, "/opt/trn_rl_repo")

import numpy as np

P = 128
EMB = 128
N_CORES = 8
N_USERS = 65536
CH = 25600  # item-table quarter size; local indices fit int16


def _build_program(T_qs, WPC, item_rows, user_rows):
    from concourse import bacc, mybir
    import concourse.bass as bass
    import concourse.tile as tile

    dt = mybir.dt
    Alu = mybir.AluOpType
    T_total = sum(T_qs)
    NT = WPC * T_total
    n_q = len(T_qs)
    q_off = np.concatenate([[0], np.cumsum(T_qs)]).astype(int)

    nc = bacc.Bacc(None, target_bir_lowering=False)
    item_t = nc.dram_tensor("item_table", [item_rows, EMB], dt.float32, kind="ExternalInput")
    user_t = nc.dram_tensor("user_table", [user_rows, EMB], dt.float32, kind="ExternalInput")
    beh_idx = nc.dram_tensor("beh_idx", [P, NT * 8], dt.int16, kind="ExternalInput")
    beh_cnt = nc.dram_tensor("beh_cnt", [P, NT], dt.float32, kind="ExternalInput")
    beh_loc = nc.dram_tensor("beh_loc", [P, NT], dt.float32, kind="ExternalInput")
    win_uid = nc.dram_tensor("win_uid", [P, WPC], dt.int32, kind="ExternalInput")
    win_glo = nc.dram_tensor("win_glo", [P, WPC], dt.float32, kind="ExternalInput")
    iota_in = nc.dram_tensor("iota", [P, P], dt.float32, kind="ExternalInput")
    gout = nc.dram_tensor("gout", [P, WPC * EMB], dt.float32, kind="ExternalOutput")

    with tile.TileContext(nc) as tc:
        with (
            tc.tile_pool(name="meta", bufs=1) as meta_tp,
            tc.tile_pool(name="gbuf", bufs=4) as gbuf_tp,
            tc.tile_pool(name="sel", bufs=3) as sel_tp,
            tc.tile_pool(name="epi", bufs=2) as epi_tp,
            tc.tile_pool(name="upsum", bufs=2, space="PSUM") as upsum_tp,
            tc.tile_pool(name="gpsum", bufs=2, space="PSUM") as gpsum_tp,
        ):
            idx_s = meta_tp.tile([P, NT * 8], dt.int16)
            nc.sync.dma_start(idx_s[:], beh_idx[:])
            cnt_s = meta_tp.tile([P, NT], dt.float32)
            nc.sync.dma_start(cnt_s[:], beh_cnt[:])
            loc_s = meta_tp.tile([P, NT], dt.float32)
            nc.sync.dma_start(loc_s[:], beh_loc[:])
            uid_s = meta_tp.tile([P, WPC], dt.int32)
            nc.sync.dma_start(uid_s[:], win_uid[:])
            glo_s = meta_tp.tile([P, WPC], dt.float32)
            nc.sync.dma_start(glo_s[:], win_glo[:])
            iota_s = meta_tp.tile([P, P], dt.float32)
            nc.sync.dma_start(iota_s[:], iota_in[:])

            for w in range(WPC):
                upsum = upsum_tp.tile([P, EMB], dt.float32)
                n_win_tiles = T_total
                done = 0
                for q in range(n_q):
                    T_q = T_qs[q]
                    if T_q == 0:
                        continue
                    t0 = w * T_total + q_off[q]
                    gb = gbuf_tp.tile([P, T_q, EMB], dt.float32, tag="gb")
                    nc.gpsimd.dma_gather(
                        gb[:],
                        item_t[q * CH : min((q + 1) * CH, item_rows), :],
                        idx_s[:, t0 * 8 : (t0 + T_q) * 8],
                        T_q * P,
                        T_q * P,
                        EMB,
                        single_packet=False,
                    )
                    for i in range(T_q):
                        t = t0 + i
                        sel = sel_tp.tile([P, P], dt.float32, tag="sel")
                        # sel[p,u] = (iota[p,u] == loc[p]) * cnt[p]
                        nc.vector.scalar_tensor_tensor(
                            out=sel[:],
                            in0=iota_s[:],
                            scalar=loc_s[:, t : t + 1],
                            in1=cnt_s[:, t : t + 1].to_broadcast([P, P]),
                            op0=Alu.is_equal,
                            op1=Alu.mult,
                        )
                        nc.tensor.matmul(
                            out=upsum[:],
                            lhsT=sel[:],
                            rhs=gb[:, i, :],
                            start=(done == 0),
                            stop=(done == n_win_tiles - 1),
                        )
                        done += 1
                # epilogue: user embeddings, per-user mult, group reduce
                ue = epi_tp.tile([P, EMB], dt.float32, tag="ue")
                nc.gpsimd.indirect_dma_start(
                    out=ue[:],
                    out_offset=None,
                    in_=user_t[:],
                    in_offset=bass.IndirectOffsetOnAxis(ap=uid_s[:, w : w + 1], axis=0),
                )
                pu = epi_tp.tile([P, EMB], dt.float32, tag="pu")
                nc.vector.tensor_tensor(out=pu[:], in0=upsum[:], in1=ue[:], op=Alu.mult)
                gsel = epi_tp.tile([P, P], dt.float32, tag="gsel")
                nc.vector.tensor_scalar(
                    out=gsel[:],
                    in0=iota_s[:],
                    scalar1=glo_s[:, w : w + 1],
                    scalar2=None,
                    op0=Alu.is_equal,
                )
                gp = gpsum_tp.tile([P, EMB], dt.float32)
                nc.tensor.matmul(out=gp[:], lhsT=gsel[:], rhs=pu[:], start=True, stop=True)
                go = epi_tp.tile([P, EMB], dt.float32, tag="go")
                nc.any.tensor_copy(out=go[:], in_=gp[:])
                nc.sync.dma_start(gout[:, w * EMB : (w + 1) * EMB], go[:])
    nc.finalize()
    return nc


def _prepare(user_ids, user_group_ids, behavior_item_ids, behavior_counts,
             behavior_user_ids, n_users, n_cores):
    n_win = n_users // P
    WPC = n_win // n_cores
    n_q = 4

    win = (behavior_user_ids.astype(np.int64)) >> 7
    q = behavior_item_ids.astype(np.int64) // CH
    gid = win * n_q + q
    # sort by item id within each (window, quarter) run: descriptor streams
    # walk the table monotonically, improving HBM page locality
    order = np.lexsort((behavior_item_ids, q, win))
    gid_s = gid[order]
    counts = np.bincount(gid, minlength=n_win * n_q)
    starts = np.concatenate([[0], np.cumsum(counts)[:-1]])
    j = np.arange(len(win), dtype=np.int64) - starts[gid_s]

    T_q_per_win = ((counts.reshape(n_win, n_q) + P - 1) // P)
    T_qs = T_q_per_win.max(axis=0).astype(int)          # uniform per quarter
    T_total = int(T_qs.sum())
    q_off = np.concatenate([[0], np.cumsum(T_qs)]).astype(np.int64)
    NT = WPC * T_total

    item_s = behavior_item_ids[order].astype(np.int64)
    cnt_s = behavior_counts[order]
    loc_s = (behavior_user_ids[order] & 127).astype(np.float32)

    core = win[order] // WPC
    w_local = win[order] % WPC
    t_glob = w_local * T_total + q_off[q[order]] + (j // P)   # tile within core
    p_in = j % P

    beh_cnt = np.zeros((n_cores, P, NT), np.float32)
    beh_loc = np.zeros((n_cores, P, NT), np.float32)
    flat = core * (P * NT) + p_in * NT + t_glob
    beh_cnt.reshape(-1)[flat] = cnt_s
    beh_loc.reshape(-1)[flat] = loc_s

    # int16 gather-index plane: [cores, 128, NT*8]; batch-local index j sits at
    # row (p%16) [replicated x8], col t*8 + p//16
    beh_idx = np.zeros((n_cores, 16, NT * 8), np.int16)
    col = t_glob * 8 + p_in // 16
    flat_i = core * (16 * NT * 8) + (p_in % 16) * (NT * 8) + col
    beh_idx.reshape(-1)[flat_i] = (item_s - q[order] * CH).astype(np.int16)
    beh_idx = np.tile(beh_idx, (1, 8, 1))

    win_uid = np.ascontiguousarray(
        user_ids.reshape(n_cores, WPC, P).transpose(0, 2, 1)).astype(np.int32)

    g = user_group_ids.astype(np.int64)
    change = np.empty(n_users, np.int64)
    change[0] = 1
    change[1:] = (g[1:] != g[:-1]).astype(np.int64)
    dense = np.cumsum(change) - 1
    uniq = user_group_ids[change.astype(bool)]
    rank_base = dense.reshape(n_win, P)[:, 0]
    local_rank = (dense - np.repeat(rank_base, P)).astype(np.float32)
    win_glo = np.ascontiguousarray(
        local_rank.reshape(n_cores, WPC, P).transpose(0, 2, 1))
    count_w = dense.reshape(n_win, P)[:, -1] - rank_base + 1

    iota = np.broadcast_to(np.arange(P, dtype=np.float32), (P, P)).copy()
    return dict(beh_idx=beh_idx, beh_cnt=beh_cnt, beh_loc=beh_loc,
                win_uid=win_uid, win_glo=win_glo, iota=iota,
                T_qs=tuple(int(x) for x in T_qs), NT=NT, WPC=WPC,
                uniq=uniq, rank_base=rank_base, count_w=count_w, n_win=n_win)


def _combine(gouts, meta, num_groups):
    WPC = meta["WPC"]
    out = np.zeros((num_groups, EMB), np.float32)
    uniq, rank_base, count_w = meta["uniq"], meta["rank_base"], meta["count_w"]
    for c in range(len(gouts)):
        slabs = gouts[c].reshape(P, WPC, EMB)
        for wl in range(WPC):
            w = c * WPC + wl
            k = int(count_w[w])
            tgt = uniq[rank_base[w] : rank_base[w] + k]
            np.add.at(out, tgt, slabs[:k, wl, :])
    return out


_CACHE = {}


def kernel(user_ids, user_group_ids, behavior_item_ids, behavior_counts,
           behavior_user_ids, user_table, item_table, num_groups):
    from concourse.bass_utils import run_bass_kernel_spmd

    user_ids = np.asarray(user_ids)
    user_group_ids = np.asarray(user_group_ids)
    behavior_item_ids = np.asarray(behavior_item_ids)
    behavior_counts = np.asarray(behavior_counts, dtype=np.float32)
    behavior_user_ids = np.asarray(behavior_user_ids)
    user_table = np.asarray(user_table, dtype=np.float32)
    item_table = np.asarray(item_table, dtype=np.float32)
    n_groups = int(num_groups)

    meta = _prepare(user_ids, user_group_ids, behavior_item_ids,
                    behavior_counts, behavior_user_ids, N_USERS, N_CORES)

    user_table_z = user_table.copy()
    user_table_z[0] = 0.0

    key = (meta["T_qs"], meta["WPC"], item_table.shape[0], user_table.shape[0])
    if key not in _CACHE:
        _CACHE[key] = _build_program(list(meta["T_qs"]), meta["WPC"],
                                     item_table.shape[0], user_table.shape[0])
    nc = _CACHE[key]

    in_maps = []
    for c in range(N_CORES):
        in_maps.append({
            "item_table": item_table,
            "user_table": user_table_z,
            "beh_idx": meta["beh_idx"][c],
            "beh_cnt": meta["beh_cnt"][c],
            "beh_loc": meta["beh_loc"][c],
            "win_uid": meta["win_uid"][c],
            "win_glo": meta["win_glo"][c],
            "iota": meta["iota"],
        })

    res = run_bass_kernel_spmd(nc, in_maps, core_ids=list(range(N_CORES)))
    gouts = [res.results[c]["gout"] for c in range(N_CORES)]
    return _combine(gouts, meta, n_groups)
